# revision 5
# baseline (speedup 1.0000x reference)
"""Binarized 3x3 conv (BinaryConnect) on 8 Trainium2 NeuronCores.

Problem: y = conv2d(x, sign(w), stride=1, pad=1) + bias
  x: (32, 256, 56, 56) f32, w: (256, 256, 3, 3) f32, bias: (256,) f32
  out: (32, 256, 56, 56) f32

Strategy (data-parallel over batch, 4 images/core):
  - Host: binarize weights (sign -> exactly representable in bf16),
    cast x to bf16, zero-pad each 56x56 plane to 58x58 (halo) and
    flatten to a padded pitch-58 layout so every conv tap (r,s) is a
    single contiguous shift of the same SBUF buffer.
  - Device: implicit GEMM. For each image / output-channel group /
    8-row block: accumulate 18 matmuls (9 taps x 2 input-channel
    groups) of [K=128c x M=128k] @ [128c x N=464px] into one PSUM
    bank, then evict with a fused per-channel bias add + crop of the
    pitch-58 garbage columns, and DMA the packed rows to HBM.
"""

import numpy as np
import ml_dtypes

import concourse.bass as bass
import concourse.bacc as bacc
import concourse.mybir as mybir
from concourse.tile import TileContext
from concourse.bass_utils import run_bass_kernel_spmd

# problem constants (hardcoded per harness contract)
N_IMG = 32
C = 256  # input channels
K = 256  # output channels
H = W = 56
HP = WP = 58  # padded
R = S = 3
N_CORES = 8
IMG_PER_CORE = N_IMG // N_CORES

L_PLANE = HP * WP  # 3364
L_PAD = L_PLANE + 4  # tail zeros: taps of the garbage column read past the plane
LEAD = 2  # leading slack in SBUF so tap (r=0,s=0) offset (-1) stays in-bounds
ROWS_PER_BLK = 8
N_BLK = H // ROWS_PER_BLK  # 7
N_FREE = ROWS_PER_BLK * WP  # 464 <= 512 (one PSUM bank)
N_OUT = ROWS_PER_BLK * W  # 448 packed output elems per block

BF16 = mybir.dt.bfloat16
F32 = mybir.dt.float32

_compiled = None


def _build_bass():
    nc = bacc.Bacc()

    xp = nc.declare_dram_parameter("xp", [IMG_PER_CORE, 2, 128, L_PAD], BF16, isOutput=False)
    wt = nc.declare_dram_parameter("wt", [2, 128, R * S * K], BF16, isOutput=False)
    bi = nc.declare_dram_parameter("bi", [2, 128, 1], F32, isOutput=False)
    y = nc.declare_dram_parameter("y", [IMG_PER_CORE, 2, 128, H * W], F32, isOutput=True)

    with TileContext(nc) as tc:
        with (
            tc.tile_pool(name="wpool", bufs=1) as wpool,
            tc.tile_pool(name="xpool", bufs=2) as xpool,
            tc.tile_pool(name="opool", bufs=6) as opool,
            tc.tile_pool(name="pspool", bufs=6, space="PSUM") as pspool,
        ):
            wsb = []
            bsb = []
            for cg in range(2):
                wtile = wpool.tile([128, R * S * K], BF16, tag=f"w{cg}")
                nc.sync.dma_start(out=wtile[:], in_=wt[cg])
                wsb.append(wtile)
                btile = wpool.tile([128, 1], F32, tag=f"b{cg}")
                nc.sync.dma_start(out=btile[:], in_=bi[cg])
                bsb.append(btile)

            for n in range(IMG_PER_CORE):
                xsb = []
                for cg in range(2):
                    xtile = xpool.tile([128, LEAD + L_PAD], BF16, tag=f"x{cg}")
                    nc.sync.dma_start(out=xtile[:, LEAD : LEAD + L_PAD], in_=xp[n, cg])
                    xsb.append(xtile)

                for kg in range(2):
                    for b in range(N_BLK):
                        ps = pspool.tile([128, N_FREE], F32, tag="ps")
                        mm = 0
                        for cg in range(2):
                            for r in range(R):
                                for s in range(S):
                                    tap = r * S + s
                                    off = LEAD + b * N_FREE + r * WP + s - 1
                                    nc.tensor.matmul(
                                        ps[:],
                                        lhsT=wsb[cg][:, tap * K + kg * 128 : tap * K + kg * 128 + 128],
                                        rhs=xsb[cg][:, off : off + N_FREE],
                                        start=(mm == 0),
                                        stop=(mm == 17),
                                    )
                                    mm += 1
                        ot = opool.tile([128, N_OUT], F32, tag="o")
                        ps_v = ps.rearrange("p (h w) -> p h w", w=WP)[:, :, 1 : 1 + W]
                        ot_v = ot.rearrange("p (h w) -> p h w", w=W)
                        nc.scalar.activation(
                            ot_v,
                            ps_v,
                            mybir.ActivationFunctionType.Identity,
                            bias=bsb[kg][:],
                        )
                        nc.sync.dma_start(
                            out=y[n, kg][:, b * N_OUT : (b + 1) * N_OUT], in_=ot[:]
                        )
    nc.compile()
    return nc


def _get_compiled():
    global _compiled
    if _compiled is None:
        _compiled = _build_bass()
    return _compiled


def _prepare_inputs(x, weight, bias):
    bf16 = ml_dtypes.bfloat16
    # binarized, transposed weights: wt[c, (r*3+s)*256 + k] = sign(w[k,c,r,s])
    w_bin = np.sign(weight.astype(np.float32))
    wt = np.ascontiguousarray(np.transpose(w_bin, (1, 2, 3, 0))).reshape(C, R * S * K)
    wt = wt.astype(bf16).reshape(2, 128, R * S * K)

    # padded pitch-58 activations
    xp = np.zeros((N_IMG, C, L_PAD), dtype=bf16)
    xp_img = xp[:, :, :L_PLANE].reshape(N_IMG, C, HP, WP)
    xp_img[:, :, 1 : 1 + H, 1 : 1 + W] = x.astype(bf16)

    bi = bias.astype(np.float32).reshape(2, 128, 1)
    return xp, wt, bi


def kernel(x, weight, bias, _trace=False, _trace_kwargs=None):
    nc = _get_compiled()
    xp, wt, bi = _prepare_inputs(x, weight, bias)

    in_maps = []
    for i in range(N_CORES):
        xs = np.ascontiguousarray(
            xp[i * IMG_PER_CORE : (i + 1) * IMG_PER_CORE].reshape(
                IMG_PER_CORE, 2, 128, L_PAD
            )
        )
        in_maps.append({"xp": xs, "wt": wt, "bi": bi})

    res = run_bass_kernel_spmd(
        nc, in_maps, list(range(N_CORES)), trace=_trace, **(_trace_kwargs or {})
    )
    out = np.concatenate(
        [r["y"].reshape(IMG_PER_CORE, K, H, W) for r in res.results], axis=0
    )
    if _trace:
        return np.asarray(out, dtype=np.float32), res
    return np.asarray(out, dtype=np.float32)


# revision 9
# speedup vs baseline: 551.3902x; 551.3902x over previous
"""Binarized 3x3 conv (BinaryConnect) on 8 Trainium2 NeuronCores.

Problem: y = conv2d(x, sign(w), stride=1, pad=1) + bias
  x: (32, 256, 56, 56) f32, w: (256, 256, 3, 3) f32, bias: (256,) f32
  out: (32, 256, 56, 56) f32

Strategy (data-parallel over batch, 4 images/core):
  - Host: binarize weights (sign -> exactly representable in bf16),
    cast x to bf16, zero-pad each 56x56 plane to 58x58 (halo) and
    flatten to a padded pitch-58 layout so every conv tap (r,s) is a
    single contiguous shift of the same SBUF buffer.
  - Device: implicit GEMM. For each image / output-channel group /
    8-row block: accumulate 18 matmuls (9 taps x 2 input-channel
    groups) of [K=128c x M=128k] @ [128c x N=464px] into one PSUM
    bank, then evict with a fused per-channel bias add + crop of the
    pitch-58 garbage columns, and DMA the packed rows to HBM.
"""

import numpy as np
import ml_dtypes

import concourse.bass as bass
import concourse.bacc as bacc
import concourse.mybir as mybir
from concourse.tile import TileContext
from concourse.bass_utils import run_bass_kernel_spmd

# problem constants (hardcoded per harness contract)
N_IMG = 32
C = 256  # input channels
K = 256  # output channels
H = W = 56
HP = WP = 58  # padded
R = S = 3
N_CORES = 8
IMG_PER_CORE = N_IMG // N_CORES

L_PLANE = HP * WP  # 3364
L_PAD = L_PLANE + 4  # tail zeros: taps of the garbage column read past the plane
LEAD = 2  # leading slack in SBUF so tap (r=0,s=0) offset (-1) stays in-bounds
ROWS_PER_BLK = 8
N_BLK = H // ROWS_PER_BLK  # 7
N_FREE = ROWS_PER_BLK * WP  # 464 <= 512 (one PSUM bank)
N_OUT = ROWS_PER_BLK * W  # 448 packed output elems per block

BF16 = mybir.dt.bfloat16
F32 = mybir.dt.float32

_compiled = {}


def _build_bass(loops=1):
    nc = bacc.Bacc()

    xp = nc.declare_dram_parameter("xp", [IMG_PER_CORE, 2, 128, L_PAD], BF16, isOutput=False)
    wt = nc.declare_dram_parameter("wt", [2, 128, R * S * K], BF16, isOutput=False)
    bi = nc.declare_dram_parameter("bi", [2, 128, 1], F32, isOutput=False)
    y = nc.declare_dram_parameter("y", [IMG_PER_CORE, 2, 128, H * W], F32, isOutput=True)

    with TileContext(nc) as tc:
        with (
            tc.tile_pool(name="wpool", bufs=1) as wpool,
            tc.tile_pool(name="xpool", bufs=2) as xpool,
            tc.tile_pool(name="opool", bufs=6) as opool,
            tc.tile_pool(name="pspool", bufs=6, space="PSUM") as pspool,
        ):
            wsb = []
            bsb = []
            for cg in range(2):
                wtile = wpool.tile([128, R * S * K], BF16, tag=f"w{cg}")
                nc.sync.dma_start(out=wtile[:], in_=wt[cg])
                wsb.append(wtile)
                btile = wpool.tile([128, 1], F32, tag=f"b{cg}")
                nc.sync.dma_start(out=btile[:], in_=bi[cg])
                bsb.append(btile)

            import contextlib

            loop_cm = tc.For_i(0, loops, 1) if loops > 1 else contextlib.nullcontext()
            with loop_cm:
                _body(nc, tc, xpool, opool, pspool, xp, y, wsb, bsb)
    nc.compile()
    return nc


def _body(nc, tc, xpool, opool, pspool, xp, y, wsb, bsb):
    for n in range(IMG_PER_CORE):
        xsb = []
        for cg in range(2):
            xtile = xpool.tile([128, LEAD + L_PAD], BF16, tag=f"x{cg}")
            nc.sync.dma_start(out=xtile[:, LEAD : LEAD + L_PAD], in_=xp[n, cg])
            xsb.append(xtile)

        for kg in range(2):
            for b in range(N_BLK):
                ps = pspool.tile([128, N_FREE], F32, tag="ps")
                mm = 0
                for cg in range(2):
                    for r in range(R):
                        for s in range(S):
                            tap = r * S + s
                            off = LEAD + b * N_FREE + r * WP + s - 1
                            nc.tensor.matmul(
                                ps[:],
                                lhsT=wsb[cg][:, tap * K + kg * 128 : tap * K + kg * 128 + 128],
                                rhs=xsb[cg][:, off : off + N_FREE],
                                start=(mm == 0),
                                stop=(mm == 17),
                            )
                            mm += 1
                ot = opool.tile([128, N_OUT], F32, tag="o")
                ps_v = ps.rearrange("p (h w) -> p h w", w=WP)[:, :, 1 : 1 + W]
                ot_v = ot.rearrange("p (h w) -> p h w", w=W)
                nc.scalar.activation(
                    ot_v,
                    ps_v,
                    mybir.ActivationFunctionType.Identity,
                    bias=bsb[kg][:],
                )
                nc.sync.dma_start(
                    out=y[n, kg][:, b * N_OUT : (b + 1) * N_OUT], in_=ot[:]
                )


def _get_compiled(loops=1):
    if loops not in _compiled:
        _compiled[loops] = _build_bass(loops)
    return _compiled[loops]


def _prepare_inputs(x, weight, bias):
    bf16 = ml_dtypes.bfloat16
    # binarized, transposed weights: wt[c, (r*3+s)*256 + k] = sign(w[k,c,r,s])
    w_bin = np.sign(weight.astype(np.float32))
    wt = np.ascontiguousarray(np.transpose(w_bin, (1, 2, 3, 0))).reshape(C, R * S * K)
    wt = wt.astype(bf16).reshape(2, 128, R * S * K)

    # padded pitch-58 activations
    xp = np.zeros((N_IMG, C, L_PAD), dtype=bf16)
    xp_img = xp[:, :, :L_PLANE].reshape(N_IMG, C, HP, WP)
    xp_img[:, :, 1 : 1 + H, 1 : 1 + W] = x.astype(bf16)

    bi = bias.astype(np.float32).reshape(2, 128, 1)
    return xp, wt, bi


def kernel(x, weight, bias, _trace=False, _trace_kwargs=None):
    nc = _get_compiled()
    xp, wt, bi = _prepare_inputs(x, weight, bias)

    in_maps = []
    for i in range(N_CORES):
        xs = np.ascontiguousarray(
            xp[i * IMG_PER_CORE : (i + 1) * IMG_PER_CORE].reshape(
                IMG_PER_CORE, 2, 128, L_PAD
            )
        )
        in_maps.append({"xp": xs, "wt": wt, "bi": bi})

    res = run_bass_kernel_spmd(
        nc, in_maps, list(range(N_CORES)), trace=_trace, **(_trace_kwargs or {})
    )
    out = np.concatenate(
        [r["y"].reshape(IMG_PER_CORE, K, H, W) for r in res.results], axis=0
    )
    if _trace:
        return np.asarray(out, dtype=np.float32), res
    return np.asarray(out, dtype=np.float32)
